# revision 2
# baseline (speedup 1.0000x reference)
"""Trainium2 Bass kernel for nn_ByteSequenceEmbedder.

Packed data-parallel across 8 NeuronCores: the 16 sequences' VALID positions
(sum of pool_lengths ~= 2048 of 3072 per sequence) are packed into 8 balanced
per-core streams (~4102 cols each instead of 2x3072), cut at word boundaries.
Mid-sequence cuts carry a 2-col receptive-field overlap on each side;
sequence boundaries inside a stream get 2 zero gap cols, and the conv1
left-padding semantics are restored by multiplying x1c with a host-provided
0/1 column mask (per-core data in DRAM, so the SPMD program stays shared).

Per-core dataflow (channels-on-partitions, fp16 activations/weights):
  embed   : host-precomputed (x0 = tok_emb[tokens] + bpe marker) -> DMA
  conv0   : 3 shifted matmuls per (T-chunk, co-chunk) accumulated in PSUM,
            ReLU+bias fused into the ACT PSUM->SBUF evacuation
  highway : 2 blocks x 2 layers; 8x4 matmuls per chunk, ReLU/Sigmoid evac,
            DVE combine x' = g*(relu(h)-x)+x
  conv1   : 12 matmuls per (chunk, co-chunk) + residual add
  pool    : ragged word max-pool as masked shifted max with host-built
            additive masks (0 where word@t has len>j, else -60000)
  proj    : projection over all stream positions; host gathers word-start
            columns while unsharding (empty pools -> proj_b row)
"""
import numpy as np

import concourse.bacc as bacc
import concourse.tile as tile
import concourse.mybir as mybir

BSZ, NW, T = 16, 1024, 3072
BED, WED = 128, 512
BPE_MASK_IDX = 4
N_CORES = 8
OVL = 2               # receptive-field overlap at mid-sequence cuts
F16 = mybir.dt.float16
F32 = mybir.dt.float32
_F16_NP = np.float16
NEG = np.float32(-60000.0)

_CACHE = {}


# ---------------------------------------------------------------- packing

def _plan_packing(pool_lengths, n_streams=N_CORES):
    pl = np.asarray(pool_lengths, np.int64)
    starts = np.cumsum(pl, axis=1) - pl

    def try_pack(cap):
        streams = [[] for _ in range(n_streams)]
        zcols = [[] for _ in range(n_streams)]
        lens = [0] * n_streams
        prev_type = [None] * n_streams
        core = 0
        for b in range(BSZ):
            w = 0
            cut = False
            while w < NW:
                if pl[b, w] == 0:
                    w += 1
                    continue
                if core >= n_streams:
                    return None
                if lens[core] == 0:
                    gap, zc = 0, []
                elif prev_type[core] == 'seqend':
                    gap, zc = 2, [lens[core] + 1]
                elif cut:
                    gap, zc = 0, []
                else:
                    gap, zc = 1, [lens[core]]
                pos = lens[core] + gap
                left = OVL if cut else 0
                room = cap - pos - left
                if room < pl[b, w]:
                    core += 1
                    continue
                cum = 0
                w0 = w
                while w < NW and cum + pl[b, w] <= room:
                    cum += pl[b, w]
                    w += 1
                body0 = int(starts[b, w0])
                is_cut_r = w < NW
                right = OVL if is_cut_r else 0
                if pos + left + cum + right > cap:
                    while w > w0 and pos + left + cum + OVL > cap:
                        w -= 1
                        cum -= pl[b, w]
                    if w == w0:
                        core += 1
                        continue
                    is_cut_r = True
                    right = OVL
                s_lo = body0 - left
                s_hi = body0 + cum + right
                streams[core].append(dict(
                    b=b, w0=w0, w1=w, s_lo=s_lo, s_hi=s_hi, pos=pos))
                zcols[core].extend(zc)
                lens[core] = pos + (s_hi - s_lo)
                prev_type[core] = 'cutend' if is_cut_r else 'seqend'
                cut = is_cut_r
        return streams, zcols, lens
    lo = int(np.ceil(pl.sum() / n_streams))
    for cap in range(lo, lo + 4096):
        r = try_pack(cap)
        if r is not None:
            return r[0], r[1], cap
    raise RuntimeError("packing failed")


def _chunk_plan(cap):
    need = cap + 1  # right-context col of a full stream must be computed
    nch = max(1, int(np.ceil(need / 512)))
    w = int(np.ceil(need / nch / 8) * 8)
    if w > 512:
        nch += 1
        w = int(np.ceil(need / nch / 8) * 8)
    return nch, w


# ---------------------------------------------------------------- program

def _build_program(nch, w):
    W = nch * w
    WB = W + 8
    nc = bacc.Bacc("TRN2", target_bir_lowering=False, debug=False)

    def dram_in(name, shape, dt):
        return nc.dram_tensor(name, shape, dt, kind="ExternalInput").ap()

    w_c0 = dram_in("w_c0", [128, 3 * WED], F16)          # [ci, k*512+co]
    w_c1 = dram_in("w_c1", [128, 4 * 3 * WED], F16)      # [ci%128, (q*3+k)*512+co]
    w_hw = dram_in("w_hw", [128, 4 * 4 * 1024], F16)     # [(bl*4+q)*1024 + co_out]
    w_pr = dram_in("w_pr", [128, 4 * WED], F16)          # [q*512+co]
    b_c0 = dram_in("b_c0", [128, 4], F32)
    b_c1 = dram_in("b_c1", [128, 4], F32)
    b_hw = dram_in("b_hw", [128, 4 * 8], F32)            # [bl*8 + m]
    b_pr = dram_in("b_pr", [128, 4], F32)
    x0_in = dram_in("x0_in", [128, WB], F16)             # host-embedded stream
    amsk = dram_in("a_msk", [128, 2 * W], F16)           # pooling additive masks
    m1c = dram_in("m1c", [128, W], F16)                  # x1c zero-col mask

    out = nc.dram_tensor("out", [WED, W], F16, kind="ExternalOutput").ap()

    RELU = mybir.ActivationFunctionType.Relu
    SIGM = mybir.ActivationFunctionType.Sigmoid
    IDEN = mybir.ActivationFunctionType.Identity
    MAX = mybir.AluOpType.max
    ADD = mybir.AluOpType.add
    SUB = mybir.AluOpType.subtract
    MUL = mybir.AluOpType.mult

    with tile.TileContext(nc) as tc:
        with tc.tile_pool(name="wp", bufs=1) as wp, \
             tc.tile_pool(name="ap", bufs=1) as apool, \
             tc.tile_pool(name="tp", bufs=3) as tp, \
             tc.tile_pool(name="pp", bufs=8, space="PSUM") as pp:

            # ---- HAM warm-up: PE activity from t~0 ----
            wu = wp.tile([128, w], F16)
            nc.vector.memset(wu[:], 0)
            for _ in range(20):
                wps = pp.tile([128, w], F32, tag="ps", name="wps")
                nc.tensor.matmul(out=wps[:], lhsT=wu[:, 0:128], rhs=wu[:],
                                 start=True, stop=True)

            # ---- input/weight loads; x0 on the scalar queue (critical path),
            # weights in consumption order on the sync queue ----
            t_x0 = wp.tile([128, WB], F16)
            nc.scalar.dma_start(out=t_x0[:], in_=x0_in[:])
            t_wc0 = wp.tile([128, 3 * WED], F16)
            t_bc0 = wp.tile([128, 4], F32)
            t_whw = wp.tile([128, 4 * 4 * 1024], F16)
            t_bhw = wp.tile([128, 4 * 8], F32)
            t_wc1 = wp.tile([128, 4 * 3 * WED], F16)
            t_bc1 = wp.tile([128, 4], F32)
            t_wpr = wp.tile([128, 4 * WED], F16)
            t_bpr = wp.tile([128, 4], F32)
            nc.sync.dma_start(out=t_wc0[:], in_=w_c0[:])
            nc.sync.dma_start(out=t_bc0[:], in_=b_c0[:])
            # highway weights by layer so hw0l0 can start early
            for bl in range(4):
                nc.sync.dma_start(out=t_whw[:, bl * 4096:(bl + 1) * 4096],
                                  in_=w_hw[:, bl * 4096:(bl + 1) * 4096])
                if bl == 0:
                    nc.sync.dma_start(out=t_bhw[:], in_=b_hw[:])
            nc.sync.dma_start(out=t_wc1[:], in_=w_c1[:])
            nc.sync.dma_start(out=t_bc1[:], in_=b_c1[:])
            nc.sync.dma_start(out=t_wpr[:], in_=w_pr[:])
            nc.sync.dma_start(out=t_bpr[:], in_=b_pr[:])
            t_m1 = wp.tile([128, W], F16)
            t_am = wp.tile([128, 2 * W], F16)
            nc.scalar.dma_start(out=t_m1[:], in_=m1c[:])
            nc.scalar.dma_start(out=t_am[:], in_=amsk[:])

            def act_buf(tag):
                b = apool.tile([128, 4 * WB], F16, tag=tag, name=tag)
                for q in range(4):
                    nc.vector.memset(b[:, q * WB:q * WB + 1], 0)
                    nc.vector.memset(b[:, q * WB + 1 + W:(q + 1) * WB], 0)
                return b

            scope = nc.named_scope

            # ---------- conv0 ----------
            with scope("conv0"):
                x1 = act_buf("actA")
                for n in range(nch):
                    for m in range(4):
                        ps = pp.tile([128, w], F32, tag="ps", name="ps")
                        for k in range(3):
                            nc.tensor.matmul(
                                out=ps[:],
                                lhsT=t_wc0[:, k * WED + m * 128:k * WED + (m + 1) * 128],
                                rhs=t_x0[:, n * w + k:n * w + k + w],
                                start=(k == 0), stop=(k == 2))
                        nc.scalar.activation(
                            out=x1[:, m * WB + 1 + n * w:m * WB + 1 + (n + 1) * w],
                            in_=ps[:], func=RELU, bias=t_bc0[:, m:m + 1], scale=1.0)

            def highway_layer(X, Y, bl):
                for n in range(nch):
                    pss = []
                    for m in range(8):
                        ps = pp.tile([128, w], F32, tag="ps", name="ps")
                        for q in range(4):
                            base = (bl * 4 + q) * 1024 + m * 128
                            nc.tensor.matmul(
                                out=ps[:], lhsT=t_whw[:, base:base + 128],
                                rhs=X[:, q * WB + 1 + n * w:q * WB + 1 + (n + 1) * w],
                                start=(q == 0), stop=(q == 3))
                        pss.append(ps)
                    for c in range(4):
                        xs = X[:, c * WB + 1 + n * w:c * WB + 1 + (n + 1) * w]
                        h_t = tp.tile([128, w], F16, tag="h", name="h_t")
                        g_t = tp.tile([128, w], F16, tag="g", name="g_t")
                        d_t = tp.tile([128, w], F16, tag="d", name="d_t")
                        nc.scalar.activation(out=h_t[:], in_=pss[c][:], func=RELU,
                                             bias=t_bhw[:, bl * 8 + c:bl * 8 + c + 1],
                                             scale=1.0)
                        nc.scalar.activation(out=g_t[:], in_=pss[4 + c][:], func=SIGM,
                                             bias=t_bhw[:, bl * 8 + 4 + c:bl * 8 + 4 + c + 1],
                                             scale=1.0)
                        nc.vector.tensor_tensor(out=d_t[:], in0=h_t[:], in1=xs, op=SUB)
                        nc.vector.tensor_tensor(out=d_t[:], in0=d_t[:], in1=g_t[:], op=MUL)
                        ys = Y[:, c * WB + 1 + n * w:c * WB + 1 + (n + 1) * w]
                        nc.vector.tensor_tensor(out=ys, in0=d_t[:], in1=xs, op=ADD)

            with scope("hw0l0"):
                x1b = act_buf("actB")
                highway_layer(x1, x1b, 0)
            with scope("hw0l1"):
                x1c = act_buf("actC")
                highway_layer(x1b, x1c, 1)

            # conv1 left-padding semantics: zero the masked columns
            with scope("m1c"):
                for n in range(nch):
                    for q in range(4):
                        s = x1c[:, q * WB + 1 + n * w:q * WB + 1 + (n + 1) * w]
                        nc.vector.tensor_tensor(
                            out=s, in0=s, in1=t_m1[:, n * w:(n + 1) * w], op=MUL)

            # ---------- conv1 (+residual) ----------
            with scope("conv1"):
                x2p = act_buf("actA")
                for n in range(nch):
                    for m in range(4):
                        ps = pp.tile([128, w], F32, tag="ps", name="ps")
                        i = 0
                        for q in range(4):
                            for k in range(3):
                                lhs = t_wc1[:, (q * 3 + k) * WED + m * 128:
                                            (q * 3 + k) * WED + (m + 1) * 128]
                                nc.tensor.matmul(
                                    out=ps[:], lhsT=lhs,
                                    rhs=x1c[:, q * WB + n * w + k:q * WB + n * w + k + w],
                                    start=(i == 0), stop=(i == 11))
                                i += 1
                        r_t = tp.tile([128, w], F16, tag="h", name="r_t")
                        nc.scalar.activation(out=r_t[:], in_=ps[:], func=RELU,
                                             bias=t_bc1[:, m:m + 1], scale=1.0)
                        xs = x1c[:, m * WB + 1 + n * w:m * WB + 1 + (n + 1) * w]
                        nc.vector.tensor_tensor(
                            out=x2p[:, m * WB + 1 + n * w:m * WB + 1 + (n + 1) * w],
                            in0=r_t[:], in1=xs, op=ADD)

            with scope("hw1l0"):
                x2b = act_buf("actB")
                highway_layer(x2p, x2b, 2)
            with scope("hw1l1"):
                x2 = act_buf("actC")
                highway_layer(x2b, x2, 3)

            # ---------- ragged max pool + projection ----------
            with scope("poolproj"):
                msel = apool.tile([128, 4 * WB], F16, tag="actA", name="msel")
                for n in range(nch):
                    lo, hi = n * w, (n + 1) * w
                    for c in range(4):
                        base = c * WB + 1
                        s1 = tp.tile([128, w], F16, tag="s1", name="s1")
                        s2 = tp.tile([128, w], F16, tag="s2", name="s2")
                        nc.vector.tensor_tensor(out=s1[:], in0=x2[:, base + 1 + lo:base + 1 + hi],
                                                in1=t_am[:, lo:hi], op=ADD)
                        nc.vector.tensor_tensor(out=s2[:], in0=x2[:, base + 2 + lo:base + 2 + hi],
                                                in1=t_am[:, W + lo:W + hi], op=ADD)
                        nc.vector.tensor_tensor(out=s1[:], in0=s1[:], in1=s2[:], op=MAX)
                        nc.vector.tensor_tensor(out=msel[:, c * WB + lo:c * WB + hi],
                                                in0=s1[:], in1=x2[:, base + lo:base + hi],
                                                op=MAX)
                    for m in range(4):
                        ps = pp.tile([128, w], F32, tag="ps", name="ps")
                        for q in range(4):
                            nc.tensor.matmul(
                                out=ps[:],
                                lhsT=t_wpr[:, q * WED + m * 128:q * WED + (m + 1) * 128],
                                rhs=msel[:, q * WB + lo:q * WB + hi],
                                start=(q == 0), stop=(q == 3))
                        o_t = tp.tile([128, w], F16, tag="o", name="o_t", bufs=4)
                        nc.scalar.activation(out=o_t[:], in_=ps[:], func=IDEN,
                                             bias=t_bpr[:, m:m + 1], scale=1.0)
                        nc.sync.dma_start(out=out[m * 128:(m + 1) * 128, lo:hi], in_=o_t[:])

    nc.compile()
    return nc


# ---------------------------------------------------------------- host prep

def _prep_inputs(inputs):
    pl = np.asarray(inputs["pool_lengths"], np.int64)
    toks = np.asarray(inputs["byte_tokens"], np.int64)
    bpe = np.asarray(inputs["bpe_mask"], bool)
    emb = np.asarray(inputs["tok_emb"], np.float32)
    starts = np.cumsum(pl, axis=1) - pl

    streams, zcols, cap = _plan_packing(pl)
    nch, w = _chunk_plan(cap)
    W = nch * w
    WB = W + 8

    def f16(x):
        return np.ascontiguousarray(np.asarray(x, np.float32).astype(_F16_NP))

    conv0_W = np.asarray(inputs["conv0_W"], np.float32)   # [3,128,512]
    conv1_W = np.asarray(inputs["conv1_W"], np.float32)   # [3,512,512]
    hw0_W = np.asarray(inputs["hw0_W"], np.float32)       # [2,1024,512]
    hw1_W = np.asarray(inputs["hw1_W"], np.float32)
    proj_W = np.asarray(inputs["proj_W"], np.float32)     # [512,512]

    w_c0 = f16(conv0_W.transpose(1, 0, 2).reshape(128, 3 * WED))
    w_c1 = f16(conv1_W.transpose(1, 0, 2).reshape(4, 128, 3, WED)
               .transpose(1, 0, 2, 3).reshape(128, 4 * 3 * WED))
    whw = np.empty((128, 16, 1024), np.float32)
    for bl, (blk, lay) in enumerate(((hw0_W, 0), (hw0_W, 1), (hw1_W, 0), (hw1_W, 1))):
        wt = blk[lay].T  # [512, 1024]
        for q in range(4):
            whw[:, bl * 4 + q, :] = wt[q * 128:(q + 1) * 128]
    w_hw = f16(whw.reshape(128, 16 * 1024))
    w_pr = f16(proj_W.T.reshape(4, 128, WED).transpose(1, 0, 2).reshape(128, 4 * WED))

    def colchunks(b):
        return np.ascontiguousarray(np.asarray(b, np.float32).reshape(4, 128).T)

    b_c0 = colchunks(inputs["conv0_b"])
    b_c1 = colchunks(inputs["conv1_b"])
    bhw = np.empty((128, 4, 8), np.float32)
    for bl, (blk, lay) in enumerate((("hw0_b", 0), ("hw0_b", 1), ("hw1_b", 0), ("hw1_b", 1))):
        b = np.asarray(inputs[blk], np.float32)[lay]
        bhw[:, bl, 0:4] = b[:512].reshape(4, 128).T
        bhw[:, bl, 4:8] = b[512:1024].reshape(4, 128).T
    b_hw = np.ascontiguousarray(bhw.reshape(128, 32))
    b_pr = colchunks(inputs["proj_b"])

    shared = dict(w_c0=w_c0, w_c1=w_c1, w_hw=w_hw, w_pr=w_pr,
                  b_c0=b_c0, b_c1=b_c1, b_hw=b_hw, b_pr=b_pr)

    in_maps = []
    gathers = []
    for core in range(N_CORES):
        m = dict(shared)
        x0 = np.zeros((128, WB), np.float32)
        a12 = np.full((2, W), NEG, np.float32)
        msk = np.ones(W, np.float32)
        gb, gw, gc = [], [], []
        for fr in streams[core]:
            b, s_lo, s_hi, pos = fr["b"], fr["s_lo"], fr["s_hi"], fr["pos"]
            fl = s_hi - s_lo
            tt = toks[b, s_lo:s_hi]
            x = emb[tt] + np.where(bpe[b, s_lo:s_hi, None], emb[BPE_MASK_IDX][None, :], 0.0)
            x0[:, 1 + pos:1 + pos + fl] = x.T
            plw = pl[b, fr["w0"]:fr["w1"]]
            st = starts[b, fr["w0"]:fr["w1"]]
            cols = pos + (st - s_lo)
            a12[0, cols[plw > 1]] = 0.0
            a12[1, cols[plw > 2]] = 0.0
            nz = plw > 0
            gb.extend([b] * int(nz.sum()))
            gw.extend(np.arange(fr["w0"], fr["w1"])[nz].tolist())
            gc.extend(cols[nz].tolist())
        for z in zcols[core]:
            msk[z] = 0.0
        m["x0_in"] = x0.astype(_F16_NP)
        m["a_msk"] = np.ascontiguousarray(
            np.broadcast_to(a12.reshape(1, 2 * W), (128, 2 * W)).astype(_F16_NP))
        m["m1c"] = np.ascontiguousarray(
            np.broadcast_to(msk[None, :], (128, W)).astype(_F16_NP))
        in_maps.append(m)
        gathers.append((np.asarray(gb), np.asarray(gw), np.asarray(gc)))
    meta = dict(gathers=gathers, nch=nch, w=w)
    return in_maps, meta


def kernel(**inputs) -> np.ndarray:
    from concourse.bass_utils import run_bass_kernel_spmd

    in_maps, meta = _prep_inputs(inputs)
    key = (meta["nch"], meta["w"])
    if _CACHE.get("key") != key:
        _CACHE["nc"] = _build_program(*key)
        _CACHE["key"] = key
    nc = _CACHE["nc"]

    res = run_bass_kernel_spmd(nc, in_maps, list(range(N_CORES)))

    proj_b = np.asarray(inputs["proj_b"], np.float32)
    full = np.empty((BSZ, NW, WED), np.float32)
    full[:] = proj_b[None, None, :]
    for core in range(N_CORES):
        o = np.asarray(res.results[core]["out"], np.float32)  # [512, W]
        gb, gw, gc = meta["gathers"][core]
        if len(gb):
            full[gb, gw] = o[:, gc].T
    return full
